# revision 11
# baseline (speedup 1.0000x reference)
"""NodeGLAM (BN -> Linear -> TAGConv(K=3) -> Linear -> TAGConv(K=3) -> concat
-> MLP heads) on 8 Trainium2 NeuronCores.

Sharding: nodes (dim 0) split 8 ways; each core owns the edges whose
destination falls in its node range.  Per message-passing hop the cores
AllGather the (bf16) scaled activations, then use indirect DMA gathers +
one-hot selection matmuls to compute the segment sums for their own nodes.
Dense layers run feature-major (features on partitions, nodes on the free
axis) so weight matrices are used directly as the stationary matmul operand.
"""

import numpy as np
import ml_dtypes

import concourse.bass as bass
import concourse.bacc as bacc
import concourse.mybir as mybir
import concourse.tile as tile
from concourse.bass_utils import run_bass_kernel_spmd
from concourse.library_config import mlp as mlp_library

F32 = mybir.dt.float32
BF16 = mybir.dt.bfloat16
I16 = mybir.dt.int16

CORES = 8
P = 128
EPS = 1e-5

bf = ml_dtypes.bfloat16


def cdiv(a, b):
    return (a + b - 1) // b


def roundup(a, b):
    return cdiv(a, b) * b


# ---------------------------------------------------------------------------
# Host-side preprocessing: edge grouping / padding / index layout
# ---------------------------------------------------------------------------

def prep_graph(edge_index, N):
    """Group edges by (dest core, dest block of 128, source quarter), pad each
    group to a multiple of 128 slots (uniform across cores), and build the
    per-core gather-index / local-column arrays in the layouts the device
    kernel consumes."""
    NCN = N // CORES            # nodes per core
    NB = cdiv(NCN, P)           # dest blocks per core
    QR = N // 4                 # rows per source quarter (int16 index range)
    assert QR <= 32767

    row = edge_index[0].astype(np.int64)
    col = edge_index[1].astype(np.int64)
    E = row.shape[0]

    deg = np.bincount(col, minlength=N)
    dis = np.where(deg > 0, 1.0 / np.sqrt(np.maximum(deg, 1.0)), 0.0).astype(
        np.float32
    )

    core = col // NCN
    lcol = col % NCN
    blk = lcol // P
    lc = lcol % P
    q = row // QR
    lr = row % QR

    G = NB * 4
    gkey = (core * G) + blk * 4 + q
    order = np.argsort(gkey, kind="stable")
    gkey_s = gkey[order]
    counts = np.bincount(gkey, minlength=CORES * G).reshape(CORES, NB, 4)

    Lmax = counts.max(axis=0)                      # [NB, 4]
    Lbq = np.where(Lmax > 0, np.maximum(roundup(1, P), 0), 0)
    Lbq = (np.ceil(Lmax / P).astype(np.int64) * P)  # 0 stays 0
    assert (counts <= Lbq[None]).all()

    offs = np.zeros(G + 1, np.int64)
    offs[1:] = np.cumsum(Lbq.reshape(-1))
    S = int(offs[-1])
    assert S % P == 0 and S > 0

    # slot position of every edge
    starts = np.zeros(CORES * G + 1, np.int64)
    starts[1:] = np.cumsum(counts.reshape(-1))
    rank = np.arange(E) - np.repeat(starts[:-1], counts.reshape(-1))
    core_s = core[order]
    g_within = gkey_s % G
    slot = offs[g_within] + rank

    lr_slots = np.zeros((CORES, S), np.int16)
    lc_slots = np.full((CORES, S), 200.0, np.float32)  # 200 => one-hot row of 0
    lr_slots[core_s, slot] = lr[order].astype(np.int16)
    lc_slots[core_s, slot] = lc[order].astype(np.float32)

    # gather-index layout: idx j -> partition j%16, column j//16, replicated 8x
    gidx = lr_slots.reshape(CORES, S // 16, 16).transpose(0, 2, 1)
    gidx = np.ascontiguousarray(np.tile(gidx, (1, 8, 1)))     # [CORES,128,S/16]
    # lc layout mirrors gather output: slot j -> partition j%128, col j//128
    lcarr = lc_slots.reshape(CORES, S // P, P).transpose(0, 2, 1)
    lcarr = np.ascontiguousarray(lcarr).astype(bf)            # [CORES,128,S/128]

    # dis node-major per block: [CORES, 128, NB]
    dn = np.zeros((CORES, NB * P), np.float32)
    dn[:, :NCN] = dis.reshape(CORES, NCN)
    dis_nd = np.ascontiguousarray(dn.reshape(CORES, NB, P).transpose(0, 2, 1))

    cfg = dict(
        N=N, E=E, NCN=NCN, NB=NB, QR=QR, S=S,
        L=Lbq.astype(int).tolist(),
    )
    return cfg, gidx, lcarr, dis_nd


# ---------------------------------------------------------------------------
# Bass program
# ---------------------------------------------------------------------------

def build_bass(cfg):
    N = cfg["N"]
    NCN = cfg["NCN"]
    NB = cfg["NB"]
    QR = cfg["QR"]
    S = cfg["S"]
    L = cfg["L"]          # [NB][4] padded group sizes (multiples of 128, or 0)
    NBP = NB * P

    nc = bacc.Bacc("TRN2", target_bir_lowering=False, num_devices=CORES)
    rg = [list(range(CORES))]

    # ---------------- kernel I/O ----------------
    xT = nc.dram_tensor("xT", [P, NCN], F32, kind="ExternalInput")
    gidx_d = nc.dram_tensor("gidx", [P, S // 16], I16, kind="ExternalInput")
    lc_d = nc.dram_tensor("lc", [P, S // P], BF16, kind="ExternalInput")
    iota_d = nc.dram_tensor("iota", [P, P], BF16, kind="ExternalInput")
    idbf_d = nc.dram_tensor("idbf", [P, P], BF16, kind="ExternalInput")
    idf32_d = nc.dram_tensor("idf32", [P, P], F32, kind="ExternalInput")
    disnd_d = nc.dram_tensor("dis_nd", [P, NB], F32, kind="ExternalInput")

    wname = [
        ("w1", [P, P], BF16), ("b1", [P, 1], F32),
        ("tag1_w", [4, P, P], BF16), ("tag1_b", [P, 1], F32),
        ("w2", [P, P], BF16), ("b2", [P, 1], F32),
        ("tag2_w", [4, P, P], BF16), ("tag2_b", [P, 1], F32),
        ("w5a", [P, P], BF16), ("w5b", [P, P], BF16), ("b5", [P, 1], F32),
        ("w6", [P, 64], BF16), ("b6", [64, 1], F32),
        ("wc", [64, 8], BF16), ("bc", [8, 1], F32),
        ("bn_gamma", [P, 1], F32), ("bn_beta", [P, 1], F32),
    ]
    wd = {nm: nc.dram_tensor(nm, sh, dt, kind="ExternalInput")
          for nm, sh, dt in wname}

    a_out = nc.dram_tensor("a_out", [NCN, 64], F32, kind="ExternalOutput")
    cl_out = nc.dram_tensor("cl_out", [NCN, 8], F32, kind="ExternalOutput")

    # ---------------- internal DRAM ----------------
    xn_dram = nc.dram_tensor("xn_dram", [P, NBP], BF16, kind="Internal")
    shards = [nc.dram_tensor(f"shard{h}", [NCN, P], BF16, kind="Internal")
              for h in range(6)]
    tables = [nc.dram_tensor(f"table{h}", [N, P], BF16, kind="Internal",
                             addr_space="Shared")
              for h in range(6)]
    bn_in = nc.dram_tensor("bn_in", [P, 2], F32, kind="Internal")
    bn_out = nc.dram_tensor("bn_out", [P, 2], F32, kind="Internal",
                            addr_space="Shared")

    def nbn(b):                      # valid node count of block b
        return min(P, NCN - b * P)

    from contextlib import ExitStack
    with tile.TileContext(nc) as tc, ExitStack() as ctx:
        nc.gpsimd.load_library(mlp_library)

        cpool = ctx.enter_context(tc.tile_pool(name="consts", bufs=1))
        slabp = ctx.enter_context(tc.tile_pool(name="slabs", bufs=1))
        wpool = ctx.enter_context(tc.tile_pool(name="weights", bufs=1))
        lp = ctx.enter_context(tc.tile_pool(name="loads", bufs=3))
        gp = ctx.enter_context(tc.tile_pool(name="gather", bufs=4))
        sp = ctx.enter_context(tc.tile_pool(name="sel", bufs=4))
        ep = ctx.enter_context(tc.tile_pool(name="evac", bufs=4))
        pp = ctx.enter_context(tc.tile_pool(name="psum", bufs=2, space="PSUM"))
        tp = ctx.enter_context(tc.tile_pool(name="tpsum", bufs=2, space="PSUM"))
        dp = ctx.enter_context(tc.tile_pool(name="dpsum", bufs=2, space="PSUM"))
        stat = ctx.enter_context(tc.tile_pool(name="stats", bufs=1))

        # ---- constants into SBUF ----
        zero_c = cpool.tile([P, 1], F32, tag="zero_c")
        nc.vector.memset(zero_c[:], 0.0)
        nc.const_aps.aps[(F32, 0.0)] = zero_c[:]
        eps_c = cpool.tile([P, 1], F32, tag="eps_c")
        nc.vector.memset(eps_c[:], EPS)

        gidx_sb = cpool.tile([P, S // 16], I16, tag="gidx_sb")
        nc.sync.dma_start(out=gidx_sb[:], in_=gidx_d[:])
        lc_sb = cpool.tile([P, S // P], BF16, tag="lc_sb")
        nc.sync.dma_start(out=lc_sb[:], in_=lc_d[:])
        iota_sb = cpool.tile([P, P], BF16, tag="iota_sb")
        nc.sync.dma_start(out=iota_sb[:], in_=iota_d[:])
        idbf_sb = cpool.tile([P, P], BF16, tag="idbf_sb")
        nc.sync.dma_start(out=idbf_sb[:], in_=idbf_d[:])
        idf32_sb = cpool.tile([P, P], F32, tag="idf32_sb")
        nc.sync.dma_start(out=idf32_sb[:], in_=idf32_d[:])
        disnd_sb = cpool.tile([P, NB], F32, tag="disnd_sb")
        nc.sync.dma_start(out=disnd_sb[:], in_=disnd_d[:])

        wsb = {}
        for nm, sh, dt in wname:
            if nm in ("tag1_w", "tag2_w"):
                tl = []
                for k in range(4):
                    t = wpool.tile([P, P], dt, tag=f"w_{nm}{k}")
                    nc.sync.dma_start(out=t[:], in_=wd[nm][k])
                    tl.append(t)
                wsb[nm] = tl
            else:
                t = wpool.tile(sh, dt, tag=f"w_{nm}")
                nc.sync.dma_start(out=t[:], in_=wd[nm][:])
                wsb[nm] = t

        # activation slabs, feature-major [128 feats, NBP nodes] bf16
        slabA = slabp.tile([P, NBP], BF16, tag="slabA")
        slabB = slabp.tile([P, NBP], BF16, tag="slabB")
        slabC = slabp.tile([P, NBP], BF16, tag="slabC")
        slabD = slabp.tile([P, NBP], BF16, tag="slabD")

        # ---- node chunks for the dense stages ----
        chunks = []
        c0 = 0
        while c0 < NCN:
            cw = min(512, NCN - c0)
            chunks.append((c0, cw))
            c0 += cw

        # ================= BatchNorm statistics =================
        nch = len(chunks)
        sums = stat.tile([P, nch], F32, tag="sums")
        sqs = stat.tile([P, nch], F32, tag="sqs")
        for ci, (o, cw) in enumerate(chunks):
            xs = lp.tile([P, 512], F32, tag="xchunk")
            nc.sync.dma_start(out=xs[:, :cw], in_=xT[:, o:o + cw])
            nc.vector.tensor_reduce(
                out=sums[:, ci:ci + 1], in_=xs[:, :cw],
                axis=mybir.AxisListType.X, op=mybir.AluOpType.add)
            x2 = lp.tile([P, 512], F32, tag="x2chunk")
            nc.scalar.square(x2[:, :cw], xs[:, :cw])
            nc.vector.tensor_reduce(
                out=sqs[:, ci:ci + 1], in_=x2[:, :cw],
                axis=mybir.AxisListType.X, op=mybir.AluOpType.add)
        tot = stat.tile([P, 2], F32, tag="tot")
        nc.vector.tensor_reduce(out=tot[:, 0:1], in_=sums[:],
                                axis=mybir.AxisListType.X,
                                op=mybir.AluOpType.add)
        nc.vector.tensor_reduce(out=tot[:, 1:2], in_=sqs[:],
                                axis=mybir.AxisListType.X,
                                op=mybir.AluOpType.add)
        nc.sync.dma_start(out=bn_in[:], in_=tot[:])
        nc.gpsimd.collective_compute(
            "AllReduce", mybir.AluOpType.add, replica_groups=rg,
            ins=[bn_in[:]], outs=[bn_out[:]])
        tot2 = stat.tile([P, 2], F32, tag="tot2")
        nc.sync.dma_start(out=tot2[:], in_=bn_out[:])

        mu = stat.tile([P, 1], F32, tag="mu")
        nc.scalar.mul(mu[:], tot2[:, 0:1], 1.0 / N)
        msq = stat.tile([P, 1], F32, tag="msq")
        nc.scalar.mul(msq[:], tot2[:, 1:2], 1.0 / N)
        mu2 = stat.tile([P, 1], F32, tag="mu2")
        nc.vector.tensor_tensor(out=mu2[:], in0=mu[:], in1=mu[:],
                                op=mybir.AluOpType.mult)
        var = stat.tile([P, 1], F32, tag="var")
        nc.vector.tensor_tensor(out=var[:], in0=msq[:], in1=mu2[:],
                                op=mybir.AluOpType.subtract)
        sd = stat.tile([P, 1], F32, tag="sd")
        nc.scalar.activation(sd[:], var[:], mybir.ActivationFunctionType.Sqrt,
                             bias=eps_c[:])
        inv = stat.tile([P, 1], F32, tag="inv")
        nc.vector.reciprocal(inv[:], sd[:])
        scale = stat.tile([P, 1], F32, tag="scale")
        nc.vector.tensor_tensor(out=scale[:], in0=inv[:], in1=wsb["bn_gamma"][:],
                                op=mybir.AluOpType.mult)
        mus = stat.tile([P, 1], F32, tag="mus")
        nc.vector.tensor_tensor(out=mus[:], in0=mu[:], in1=scale[:],
                                op=mybir.AluOpType.mult)
        shift = stat.tile([P, 1], F32, tag="shift")
        nc.vector.tensor_tensor(out=shift[:], in0=wsb["bn_beta"][:], in1=mus[:],
                                op=mybir.AluOpType.subtract)

        # ============ normalize + first linear (h1 = gelu(xn@w1+b1)) ============
        for (o, cw) in chunks:
            xs = lp.tile([P, 512], F32, tag="xchunk")
            nc.sync.dma_start(out=xs[:, :cw], in_=xT[:, o:o + cw])
            xn = lp.tile([P, 512], BF16, tag="xnchunk")
            nc.scalar.activation(xn[:, :cw], xs[:, :cw],
                                 mybir.ActivationFunctionType.Identity,
                                 bias=shift[:], scale=scale[:])
            nc.sync.dma_start(out=xn_dram[:, o:o + cw], in_=xn[:, :cw])
            ps = dp.tile([P, 512], F32, tag="dense_ps")
            nc.tensor.matmul(ps[:, :cw], lhsT=wsb["w1"][:], rhs=xn[:, :cw],
                             start=True, stop=True)
            nc.scalar.activation(slabA[:, o:o + cw], ps[:, :cw],
                                 mybir.ActivationFunctionType.Gelu,
                                 bias=wsb["b1"][:])

        # ================= helpers =================
        def write_ds0(h_slab, shard):
            """shard <- dis * h  (node-major), from feature-major slab."""
            for b in range(NB):
                pt = tp.tile([P, P], BF16, tag="trb_ps")
                nc.tensor.transpose(pt[:], h_slab[:, b * P:(b + 1) * P],
                                    idbf_sb[:])
                ds = ep.tile([P, P], BF16, tag="ds_tile")
                nc.vector.tensor_scalar_mul(ds[:], pt[:], disnd_sb[:, b:b + 1])
                nn = nbn(b)
                nc.sync.dma_start(out=shard[b * P:b * P + nn, :],
                                  in_=ds[:nn, :])

        def allgather(shard, table):
            nc.gpsimd.collective_compute(
                "AllGather", mybir.AluOpType.bypass, replica_groups=rg,
                ins=[shard[:]], outs=[table[:]])

        def hop(table, t_slab, shard_next):
            """t_slab <- dis * (segment-sum over edges of table[row]);
            shard_next (if not None) <- dis * t_slab  (node-major)."""
            for b in range(NB):
                ntot = sum(L[b][q] for q in range(4)) // P
                assert ntot > 0
                ps = pp.tile([P, P], F32, tag="scat_ps")
                mm = 0
                off = sum(sum(L[bb]) for bb in range(b))
                for q in range(4):
                    Lq = L[b][q]
                    if Lq == 0:
                        continue
                    nt = Lq // P
                    o16 = (off // 16)
                    o128 = (off // P)
                    g = gp.tile([P, 8, P], BF16, tag="gath")
                    nc.gpsimd.dma_gather(
                        out_ap=g[:, :nt, :],
                        in_ap=table[q * QR:(q + 1) * QR, :],
                        idxs_ap=gidx_sb[:, o16:o16 + Lq // 16],
                        num_idxs=Lq, num_idxs_reg=Lq, elem_size=P)
                    sel = sp.tile([P, 8, P], BF16, tag="sel")
                    nc.vector.tensor_tensor(
                        out=sel[:, :nt, :],
                        in0=lc_sb[:, o128:o128 + nt, None].to_broadcast(
                            [P, nt, P]),
                        in1=iota_sb[:, None, :].to_broadcast([P, nt, P]),
                        op=mybir.AluOpType.is_equal)
                    for t in range(nt):
                        nc.tensor.matmul(ps[:], lhsT=sel[:, t, :],
                                         rhs=g[:, t, :],
                                         start=(mm == 0), stop=(mm == ntot - 1))
                        mm += 1
                    off += Lq
                tn = ep.tile([P, P], BF16, tag="tn_tile")
                nc.vector.tensor_scalar_mul(tn[:], ps[:], disnd_sb[:, b:b + 1])
                if shard_next is not None:
                    ds = ep.tile([P, P], BF16, tag="ds_tile")
                    nc.vector.tensor_scalar_mul(ds[:], tn[:],
                                                disnd_sb[:, b:b + 1])
                    nn = nbn(b)
                    nc.sync.dma_start(out=shard_next[b * P:b * P + nn, :],
                                      in_=ds[:nn, :])
                pt = tp.tile([P, P], BF16, tag="trb_ps")
                nc.tensor.transpose(pt[:], tn[:], idbf_sb[:])
                nc.scalar.copy(t_slab[:, b * P:(b + 1) * P], pt[:])

        def tag_out(w4, bias, slabs_in, slab_out):
            """slab_out <- gelu(sum_k W_k^T t_k + b)."""
            for b in range(NB):
                po = dp.tile([P, 512], F32, tag="dense_ps")
                for k in range(4):
                    nc.tensor.matmul(po[:, :P], lhsT=w4[k][:],
                                     rhs=slabs_in[k][:, b * P:(b + 1) * P],
                                     start=(k == 0), stop=(k == 3))
                nc.scalar.activation(slab_out[:, b * P:(b + 1) * P],
                                     po[:, :P],
                                     mybir.ActivationFunctionType.Gelu,
                                     bias=bias[:])

        def dense_inplace(w, bias, slab):
            for (o, cw) in chunks:
                ps = dp.tile([P, 512], F32, tag="dense_ps")
                nc.tensor.matmul(ps[:, :cw], lhsT=w[:], rhs=slab[:, o:o + cw],
                                 start=True, stop=True)
                nc.scalar.activation(slab[:, o:o + cw], ps[:, :cw],
                                     mybir.ActivationFunctionType.Gelu,
                                     bias=bias[:])

        # ================= TAG conv 1 =================
        write_ds0(slabA, shards[0])
        allgather(shards[0], tables[0])
        hop(tables[0], slabB, shards[1])
        allgather(shards[1], tables[1])
        hop(tables[1], slabC, shards[2])
        allgather(shards[2], tables[2])
        hop(tables[2], slabD, None)
        t1w = [wsb["tag1_w"][k] for k in range(4)]
        tag_out(t1w, wsb["tag1_b"], [slabA, slabB, slabC, slabD], slabB)

        # ================= middle linear =================
        dense_inplace(wsb["w2"], wsb["b2"], slabB)

        # ================= TAG conv 2 =================
        write_ds0(slabB, shards[3])
        allgather(shards[3], tables[3])
        hop(tables[3], slabA, shards[4])
        allgather(shards[4], tables[4])
        hop(tables[4], slabC, shards[5])
        allgather(shards[5], tables[5])
        hop(tables[5], slabD, None)
        t2w = [wsb["tag2_w"][k] for k in range(4)]
        tag_out(t2w, wsb["tag2_b"], [slabB, slabA, slabC, slabD], slabA)

        # ================= heads =================
        for b in range(NB):
            nn = nbn(b)
            xn = lp.tile([P, P], BF16, tag="xn_head")
            nc.sync.dma_start(out=xn[:], in_=xn_dram[:, b * P:(b + 1) * P])
            p5 = dp.tile([P, 512], F32, tag="dense_ps")
            nc.tensor.matmul(p5[:, :P], lhsT=wsb["w5a"][:], rhs=xn[:],
                             start=True, stop=False)
            nc.tensor.matmul(p5[:, :P], lhsT=wsb["w5b"][:],
                             rhs=slabA[:, b * P:(b + 1) * P],
                             start=False, stop=True)
            a1 = ep.tile([P, P], BF16, tag="a1")
            nc.scalar.activation(a1[:], p5[:, :P],
                                 mybir.ActivationFunctionType.Gelu,
                                 bias=wsb["b5"][:])
            p6 = tp.tile([P, P], F32, tag="tr_ps")
            nc.tensor.matmul(p6[:64, :P], lhsT=wsb["w6"][:], rhs=a1[:],
                             start=True, stop=True)
            a6 = ep.tile([64, P], F32, tag="a6")
            nc.scalar.activation(a6[:], p6[:64, :P],
                                 mybir.ActivationFunctionType.Identity,
                                 bias=wsb["b6"][:])
            g6 = ep.tile([64, P], BF16, tag="g6")
            nc.scalar.activation(g6[:], a6[:],
                                 mybir.ActivationFunctionType.Gelu)
            pc = tp.tile([P, P], F32, tag="tr_ps")
            nc.tensor.matmul(pc[:8, :P], lhsT=wsb["wc"][:], rhs=g6[:],
                             start=True, stop=True)
            cl_f = ep.tile([8, P], F32, tag="cl_f")
            nc.scalar.activation(cl_f[:], pc[:8, :P],
                                 mybir.ActivationFunctionType.Identity,
                                 bias=wsb["bc"][:])
            # softmax over the 64 features: transpose to node-major
            pa = tp.tile([P, P], F32, tag="tr_ps")
            nc.tensor.transpose(pa[:, :64], a6[:], idf32_sb[:64, :64])
            mx = ep.tile([P, 1], F32, tag="mx")
            nc.vector.tensor_reduce(out=mx[:], in_=pa[:, :64],
                                    axis=mybir.AxisListType.X,
                                    op=mybir.AluOpType.max)
            nmx = ep.tile([P, 1], F32, tag="nmx")
            nc.vector.tensor_scalar_mul(nmx[:], mx[:], -1.0)
            ex = ep.tile([P, 64], F32, tag="ex")
            nc.scalar.activation(ex[:], pa[:, :64],
                                 mybir.ActivationFunctionType.Exp,
                                 bias=nmx[:])
            sm = ep.tile([P, 1], F32, tag="sm")
            nc.vector.tensor_reduce(out=sm[:], in_=ex[:],
                                    axis=mybir.AxisListType.X,
                                    op=mybir.AluOpType.add)
            rs = ep.tile([P, 1], F32, tag="rs")
            nc.vector.reciprocal(rs[:], sm[:])
            ao = ep.tile([P, 64], F32, tag="ao")
            nc.vector.tensor_scalar_mul(ao[:], ex[:], rs[:])
            nc.sync.dma_start(out=a_out[b * P:b * P + nn, :], in_=ao[:nn, :])
            # cl to node-major
            pcl = tp.tile([P, P], F32, tag="tr_ps")
            nc.tensor.transpose(pcl[:, :8], cl_f[:], idf32_sb[:8, :8])
            co = ep.tile([P, 8], F32, tag="co")
            nc.vector.tensor_copy(out=co[:], in_=pcl[:, :8])
            nc.sync.dma_start(out=cl_out[b * P:b * P + nn, :], in_=co[:nn, :])

    nc.compile()
    return nc


# ---------------------------------------------------------------------------
# Host driver
# ---------------------------------------------------------------------------

def prep_inputs(inputs):
    x = np.asarray(inputs["x"], np.float32)
    ei = np.asarray(inputs["edge_index"])
    N = x.shape[0]
    cfg, gidx, lcarr, dis_nd = prep_graph(ei, N)
    NCN = cfg["NCN"]

    iota = np.tile(np.arange(P, dtype=np.float32)[None, :], (P, 1)).astype(bf)
    idbf = np.eye(P, dtype=np.float32).astype(bf)
    idf32 = np.eye(P, dtype=np.float32)

    w5 = np.asarray(inputs["w5"], np.float32)
    com = {
        "iota": iota, "idbf": idbf, "idf32": idf32,
        "w1": np.asarray(inputs["w1"]).astype(bf),
        "b1": np.asarray(inputs["b1"], np.float32).reshape(-1, 1),
        "tag1_w": np.asarray(inputs["tag1_w"]).astype(bf),
        "tag1_b": np.asarray(inputs["tag1_b"], np.float32).reshape(-1, 1),
        "w2": np.asarray(inputs["w2"]).astype(bf),
        "b2": np.asarray(inputs["b2"], np.float32).reshape(-1, 1),
        "tag2_w": np.asarray(inputs["tag2_w"]).astype(bf),
        "tag2_b": np.asarray(inputs["tag2_b"], np.float32).reshape(-1, 1),
        "w5a": w5[:P].astype(bf), "w5b": w5[P:].astype(bf),
        "b5": np.asarray(inputs["b5"], np.float32).reshape(-1, 1),
        "w6": np.asarray(inputs["w6"]).astype(bf),
        "b6": np.asarray(inputs["b6"], np.float32).reshape(-1, 1),
        "wc": np.asarray(inputs["wc"]).astype(bf),
        "bc": np.asarray(inputs["bc"], np.float32).reshape(-1, 1),
        "bn_gamma": np.asarray(inputs["bn_gamma"], np.float32).reshape(-1, 1),
        "bn_beta": np.asarray(inputs["bn_beta"], np.float32).reshape(-1, 1),
    }

    in_maps = []
    for c in range(CORES):
        m = dict(com)
        m["xT"] = np.ascontiguousarray(x[c * NCN:(c + 1) * NCN].T)
        m["gidx"] = gidx[c]
        m["lc"] = lcarr[c]
        m["dis_nd"] = dis_nd[c]
        in_maps.append(m)
    return cfg, in_maps


def run(inputs, trace=False):
    cfg, in_maps = prep_inputs(inputs)
    nc = build_bass(cfg)
    res = run_bass_kernel_spmd(nc, in_maps, core_ids=list(range(CORES)),
                               trace=trace)
    a = np.concatenate([r["a_out"] for r in res.results], axis=0)
    cl = np.concatenate([r["cl_out"] for r in res.results], axis=0)
    return (a, cl), res


def kernel(**inputs):
    out, _ = run(inputs, trace=False)
    return out


# revision 22
# speedup vs baseline: 2.5459x; 2.5459x over previous
"""NodeGLAM (BN -> Linear -> TAGConv(K=3) -> Linear -> TAGConv(K=3) -> concat
-> MLP heads) on 8 Trainium2 NeuronCores.

Sharding: nodes (dim 0) split 8 ways; each core owns the edges whose
destination falls in its node range.  Per message-passing hop the cores
AllGather the (bf16) scaled activations, then use indirect DMA gathers +
one-hot selection matmuls to compute the segment sums for their own nodes.
Dense layers run feature-major (features on partitions, nodes on the free
axis) so weight matrices are used directly as the stationary matmul operand.
"""

import numpy as np
import ml_dtypes

import concourse.bass as bass
import concourse.bacc as bacc
import concourse.mybir as mybir
import concourse.tile as tile
from concourse.bass_utils import run_bass_kernel_spmd
from concourse.library_config import mlp as mlp_library

F32 = mybir.dt.float32
BF16 = mybir.dt.bfloat16
I16 = mybir.dt.int16

CORES = 8
P = 128
EPS = 1e-5

bf = ml_dtypes.bfloat16


def cdiv(a, b):
    return (a + b - 1) // b


def roundup(a, b):
    return cdiv(a, b) * b


# ---------------------------------------------------------------------------
# Host-side preprocessing: edge grouping / padding / index layout
# ---------------------------------------------------------------------------

def prep_graph(edge_index, N):
    """Group edges by (dest core, dest block of 128, source quarter), pad each
    group to a multiple of 128 slots (uniform across cores), and build the
    per-core gather-index / local-column arrays in the layouts the device
    kernel consumes."""
    NCN = N // CORES            # nodes per core
    NB = cdiv(NCN, P)           # dest blocks per core
    QR = N // 4                 # rows per source quarter (int16 index range)
    assert QR <= 32767

    row = edge_index[0].astype(np.int64)
    col = edge_index[1].astype(np.int64)
    E = row.shape[0]

    deg = np.bincount(col, minlength=N)
    dis = np.where(deg > 0, 1.0 / np.sqrt(np.maximum(deg, 1.0)), 0.0).astype(
        np.float32
    )

    core = col // NCN
    lcol = col % NCN
    blk = lcol // P
    lc = lcol % P
    q = row // QR
    lr = row % QR

    G = NB * 4
    gkey = (core * G) + blk * 4 + q
    order = np.argsort(gkey, kind="stable")
    gkey_s = gkey[order]
    counts = np.bincount(gkey, minlength=CORES * G).reshape(CORES, NB, 4)

    Lmax = counts.max(axis=0)                       # [NB, 4]
    Lbq = (np.ceil(Lmax / P).astype(np.int64) * P)  # 128-padded (sel/lc/tiles)
    # Gather counts must cover the full 128-padded tile: slots never written
    # by any gather would otherwise hold stale SBUF garbage, and 0 * NaN
    # poisons the selection matmul.
    L16 = Lbq.copy()
    assert (counts <= L16[None]).all() and (L16 <= Lbq).all()

    offs = np.zeros(G + 1, np.int64)
    offs[1:] = np.cumsum(Lbq.reshape(-1))
    S = int(offs[-1])
    offs16 = np.zeros(G + 1, np.int64)
    offs16[1:] = np.cumsum(L16.reshape(-1))
    S16 = int(offs16[-1])
    assert S % P == 0 and S > 0 and S16 % 16 == 0

    # slot position of every edge
    starts = np.zeros(CORES * G + 1, np.int64)
    starts[1:] = np.cumsum(counts.reshape(-1))
    rank = np.arange(E) - np.repeat(starts[:-1], counts.reshape(-1))
    core_s = core[order]
    g_within = gkey_s % G

    # gather indices: packed at 16-padded offsets
    lr_slots = np.zeros((CORES, S16), np.int16)
    lr_slots[core_s, offs16[g_within] + rank] = lr[order].astype(np.int16)
    # local dest columns: packed at 128-padded offsets (200 => one-hot row 0)
    lc_slots = np.full((CORES, S), 200.0, np.float32)
    lc_slots[core_s, offs[g_within] + rank] = lc[order].astype(np.float32)

    # gather-index layout: idx j -> partition j%16, column j//16, replicated 8x
    gidx = lr_slots.reshape(CORES, S16 // 16, 16).transpose(0, 2, 1)
    gidx = np.ascontiguousarray(np.tile(gidx, (1, 8, 1)))   # [CORES,128,S16/16]
    # selection one-hots in gather-output layout: slot j -> partition j%128,
    # tile j//128; sel[core, p, c*128:(c+1)*128] = onehot(lc[slot c*128+p])
    lcarr = lc_slots.reshape(CORES, S // P, P).transpose(0, 2, 1)
    sel = (lcarr[:, :, :, None] == np.arange(P, dtype=np.float32)).astype(bf)
    sel = np.ascontiguousarray(sel.reshape(CORES, P, S))    # [CORES,128,S]

    # dis node-major per block: [CORES, 128, NB]
    dn = np.zeros((CORES, NB * P), np.float32)
    dn[:, :NCN] = dis.reshape(CORES, NCN)
    dis_nd = np.ascontiguousarray(dn.reshape(CORES, NB, P).transpose(0, 2, 1))

    cfg = dict(
        N=N, E=E, NCN=NCN, NB=NB, QR=QR, S=S, S16=S16,
        L=Lbq.astype(int).tolist(), L16=L16.astype(int).tolist(),
    )
    return cfg, gidx, sel, dis_nd


# ---------------------------------------------------------------------------
# Bass program
# ---------------------------------------------------------------------------

def build_bass(cfg):
    N = cfg["N"]
    NCN = cfg["NCN"]
    NB = cfg["NB"]
    QR = cfg["QR"]
    S = cfg["S"]
    S16 = cfg["S16"]
    L = cfg["L"]          # [NB][4] padded group sizes (multiples of 128, or 0)
    L16 = cfg["L16"]      # [NB][4] 16-padded gather counts
    NBP = NB * P
    NQUEUE = 4

    nc = bacc.Bacc("TRN2", target_bir_lowering=False, num_devices=CORES,
                   num_swdge_queues=NQUEUE)
    rg = [list(range(CORES))]

    # ---------------- kernel I/O ----------------
    xT = nc.dram_tensor("xT", [P, NCN], F32, kind="ExternalInput")
    gidx_d = nc.dram_tensor("gidx", [P, S16 // 16], I16, kind="ExternalInput")
    sel_d = nc.dram_tensor("sel", [P, S], BF16, kind="ExternalInput")
    idbf_d = nc.dram_tensor("idbf", [P, P], BF16, kind="ExternalInput")
    idf32_d = nc.dram_tensor("idf32", [P, P], F32, kind="ExternalInput")
    disnd_d = nc.dram_tensor("dis_nd", [P, NB], F32, kind="ExternalInput")

    wname = [
        ("w1", [P, P], BF16), ("b1", [P, 1], F32),
        ("tag1_w", [4, P, P], BF16), ("tag1_b", [P, 1], F32),
        ("w2", [P, P], BF16), ("b2", [P, 1], F32),
        ("tag2_w", [4, P, P], BF16), ("tag2_b", [P, 1], F32),
        ("w5a", [P, P], BF16), ("w5b", [P, P], BF16), ("b5", [P, 1], F32),
        ("w6", [P, 64], BF16), ("b6", [64, 1], F32),
        ("wc", [64, 8], BF16), ("bc", [8, 1], F32),
        ("bn_gamma", [P, 1], F32), ("bn_beta", [P, 1], F32),
    ]
    wd = {nm: nc.dram_tensor(nm, sh, dt, kind="ExternalInput")
          for nm, sh, dt in wname}

    a_out = nc.dram_tensor("a_out", [NCN, 64], F32, kind="ExternalOutput")
    cl_out = nc.dram_tensor("cl_out", [NCN, 8], F32, kind="ExternalOutput")

    # ---------------- internal DRAM ----------------
    xn_dram = nc.dram_tensor("xn_dram", [P, NBP], BF16, kind="Internal")
    shards = [nc.dram_tensor(f"shard{h}", [NCN, P], BF16, kind="Internal")
              for h in range(6)]
    tables = [nc.dram_tensor(f"table{h}", [N, P], BF16, kind="Internal",
                             addr_space="Shared")
              for h in range(6)]
    bn_in = nc.dram_tensor("bn_in", [P, 2], F32, kind="Internal")
    bn_out = nc.dram_tensor("bn_out", [P, 2], F32, kind="Internal",
                            addr_space="Shared")

    def nbn(b):                      # valid node count of block b
        return min(P, NCN - b * P)

    from contextlib import ExitStack
    with tile.TileContext(nc) as tc, ExitStack() as ctx:
        nc.gpsimd.load_library(mlp_library)

        cpool = ctx.enter_context(tc.tile_pool(name="consts", bufs=1))
        slabp = ctx.enter_context(tc.tile_pool(name="slabs", bufs=1))
        wpool = ctx.enter_context(tc.tile_pool(name="weights", bufs=1))
        lp = ctx.enter_context(tc.tile_pool(name="loads", bufs=2))
        gp = ctx.enter_context(tc.tile_pool(name="gather", bufs=8))
        sp = ctx.enter_context(tc.tile_pool(name="sel", bufs=8))
        ep = ctx.enter_context(tc.tile_pool(name="evac", bufs=4))
        pp = ctx.enter_context(tc.tile_pool(name="psum", bufs=2, space="PSUM"))
        tp = ctx.enter_context(tc.tile_pool(name="tpsum", bufs=2, space="PSUM"))
        dp = ctx.enter_context(tc.tile_pool(name="dpsum", bufs=2, space="PSUM"))
        stat = ctx.enter_context(tc.tile_pool(name="stats", bufs=1))

        # ---- constants into SBUF ----
        zero_c = cpool.tile([P, 1], F32, tag="zero_c")
        nc.vector.memset(zero_c[:], 0.0)
        nc.const_aps.aps[(F32, 0.0)] = zero_c[:]
        eps_c = cpool.tile([P, 1], F32, tag="eps_c")
        nc.vector.memset(eps_c[:], EPS)

        gidx_sb = cpool.tile([P, S16 // 16], I16, tag="gidx_sb")
        nc.sync.dma_start(out=gidx_sb[:], in_=gidx_d[:])
        idbf_sb = cpool.tile([P, P], BF16, tag="idbf_sb")
        nc.sync.dma_start(out=idbf_sb[:], in_=idbf_d[:])
        idf32_sb = cpool.tile([P, P], F32, tag="idf32_sb")
        nc.sync.dma_start(out=idf32_sb[:], in_=idf32_d[:])
        disnd_sb = cpool.tile([P, NB], F32, tag="disnd_sb")
        nc.sync.dma_start(out=disnd_sb[:], in_=disnd_d[:])

        wsb = {}
        for nm, sh, dt in wname:
            if nm in ("tag1_w", "tag2_w"):
                tl = []
                for k in range(4):
                    t = wpool.tile([P, P], dt, tag=f"w_{nm}{k}")
                    nc.sync.dma_start(out=t[:], in_=wd[nm][k])
                    tl.append(t)
                wsb[nm] = tl
            else:
                t = wpool.tile(sh, dt, tag=f"w_{nm}")
                nc.sync.dma_start(out=t[:], in_=wd[nm][:])
                wsb[nm] = t

        # activation slabs, feature-major [128 feats, NBP nodes] bf16
        slabA = slabp.tile([P, NBP], BF16, tag="slabA")
        slabB = slabp.tile([P, NBP], BF16, tag="slabB")
        slabC = slabp.tile([P, NBP], BF16, tag="slabC")
        slabD = slabp.tile([P, NBP], BF16, tag="slabD")

        # ---- node chunks for the dense stages ----
        chunks = []
        c0 = 0
        while c0 < NCN:
            cw = min(512, NCN - c0)
            chunks.append((c0, cw))
            c0 += cw

        # ================= BatchNorm statistics =================
        nch = len(chunks)
        sums = stat.tile([P, nch], F32, tag="sums")
        sqs = stat.tile([P, nch], F32, tag="sqs")
        for ci, (o, cw) in enumerate(chunks):
            xs = lp.tile([P, 512], F32, tag="xchunk")
            nc.sync.dma_start(out=xs[:, :cw], in_=xT[:, o:o + cw])
            nc.vector.tensor_reduce(
                out=sums[:, ci:ci + 1], in_=xs[:, :cw],
                axis=mybir.AxisListType.X, op=mybir.AluOpType.add)
            x2 = lp.tile([P, 512], F32, tag="x2chunk")
            nc.scalar.square(x2[:, :cw], xs[:, :cw])
            nc.vector.tensor_reduce(
                out=sqs[:, ci:ci + 1], in_=x2[:, :cw],
                axis=mybir.AxisListType.X, op=mybir.AluOpType.add)
        tot = stat.tile([P, 2], F32, tag="tot")
        nc.vector.tensor_reduce(out=tot[:, 0:1], in_=sums[:],
                                axis=mybir.AxisListType.X,
                                op=mybir.AluOpType.add)
        nc.vector.tensor_reduce(out=tot[:, 1:2], in_=sqs[:],
                                axis=mybir.AxisListType.X,
                                op=mybir.AluOpType.add)
        nc.sync.dma_start(out=bn_in[:], in_=tot[:])
        nc.gpsimd.collective_compute(
            "AllReduce", mybir.AluOpType.add, replica_groups=rg,
            ins=[bn_in[:]], outs=[bn_out[:]])
        tot2 = stat.tile([P, 2], F32, tag="tot2")
        nc.sync.dma_start(out=tot2[:], in_=bn_out[:])

        mu = stat.tile([P, 1], F32, tag="mu")
        nc.scalar.mul(mu[:], tot2[:, 0:1], 1.0 / N)
        msq = stat.tile([P, 1], F32, tag="msq")
        nc.scalar.mul(msq[:], tot2[:, 1:2], 1.0 / N)
        mu2 = stat.tile([P, 1], F32, tag="mu2")
        nc.vector.tensor_tensor(out=mu2[:], in0=mu[:], in1=mu[:],
                                op=mybir.AluOpType.mult)
        var = stat.tile([P, 1], F32, tag="var")
        nc.vector.tensor_tensor(out=var[:], in0=msq[:], in1=mu2[:],
                                op=mybir.AluOpType.subtract)
        sd = stat.tile([P, 1], F32, tag="sd")
        nc.scalar.activation(sd[:], var[:], mybir.ActivationFunctionType.Sqrt,
                             bias=eps_c[:])
        inv = stat.tile([P, 1], F32, tag="inv")
        nc.vector.reciprocal(inv[:], sd[:])
        scale = stat.tile([P, 1], F32, tag="scale")
        nc.vector.tensor_tensor(out=scale[:], in0=inv[:], in1=wsb["bn_gamma"][:],
                                op=mybir.AluOpType.mult)
        mus = stat.tile([P, 1], F32, tag="mus")
        nc.vector.tensor_tensor(out=mus[:], in0=mu[:], in1=scale[:],
                                op=mybir.AluOpType.mult)
        shift = stat.tile([P, 1], F32, tag="shift")
        nc.vector.tensor_tensor(out=shift[:], in0=wsb["bn_beta"][:], in1=mus[:],
                                op=mybir.AluOpType.subtract)

        # ============ normalize + first linear (h1 = gelu(xn@w1+b1)) ============
        for (o, cw) in chunks:
            xs = lp.tile([P, 512], F32, tag="xchunk")
            nc.sync.dma_start(out=xs[:, :cw], in_=xT[:, o:o + cw])
            xn = lp.tile([P, 512], BF16, tag="xnchunk")
            nc.scalar.activation(xn[:, :cw], xs[:, :cw],
                                 mybir.ActivationFunctionType.Identity,
                                 bias=shift[:], scale=scale[:])
            nc.sync.dma_start(out=xn_dram[:, o:o + cw], in_=xn[:, :cw])
            ps = dp.tile([P, 512], F32, tag="dense_ps")
            nc.tensor.matmul(ps[:, :cw], lhsT=wsb["w1"][:], rhs=xn[:, :cw],
                             start=True, stop=True)
            nc.scalar.activation(slabA[:, o:o + cw], ps[:, :cw],
                                 mybir.ActivationFunctionType.Gelu,
                                 bias=wsb["b1"][:])

        # ================= helpers =================
        def write_ds0(h_slab, shard):
            """shard <- dis * h  (node-major), from feature-major slab."""
            for b in range(NB):
                pt = tp.tile([P, P], BF16, tag="trb_ps")
                nc.tensor.transpose(pt[:], h_slab[:, b * P:(b + 1) * P],
                                    idbf_sb[:])
                ds = ep.tile([P, P], BF16, tag="ds_tile")
                nc.scalar.mul(ds[:], pt[:], disnd_sb[:, b:b + 1])
                nn = nbn(b)
                nc.sync.dma_start(out=shard[b * P:b * P + nn, :],
                                  in_=ds[:nn, :])

        def allgather(shard, table):
            nc.gpsimd.collective_compute(
                "AllGather", mybir.AluOpType.bypass, replica_groups=rg,
                ins=[shard[:]], outs=[table[:]])

        qctr = [0]

        def hop(table, t_slab, shard_next):
            """t_slab <- dis * (segment-sum over edges of table[row]);
            shard_next (if not None) <- dis * t_slab  (node-major)."""
            for b in range(NB):
                ntot = sum(L[b][q] for q in range(4)) // P
                assert ntot > 0
                ps = pp.tile([P, P], F32, tag="scat_ps")
                mm = 0
                off = sum(sum(L[bb]) for bb in range(b))
                off16 = sum(sum(L16[bb]) for bb in range(b))
                for q in range(4):
                    Lq = L[b][q]
                    Lg = L16[b][q]
                    if Lq == 0:
                        continue
                    nt = Lq // P
                    g = gp.tile([P, 8, P], BF16, tag="gath")
                    nc.gpsimd.dma_gather(
                        out_ap=g[:, :nt, :],
                        in_ap=table[q * QR:(q + 1) * QR, :],
                        idxs_ap=gidx_sb[:, off16 // 16:(off16 + Lg) // 16],
                        num_idxs=Lg, num_idxs_reg=Lg, elem_size=P,
                        queue_num=qctr[0] % NQUEUE)
                    qctr[0] += 1
                    sel = sp.tile([P, 8 * P], BF16, tag="sel")
                    nc.sync.dma_start(out=sel[:, :Lq],
                                      in_=sel_d[:, off:off + Lq])
                    for t in range(nt):
                        nc.tensor.matmul(ps[:], lhsT=sel[:, t * P:(t + 1) * P],
                                         rhs=g[:, t, :],
                                         start=(mm == 0), stop=(mm == ntot - 1))
                        mm += 1
                    off += Lq
                    off16 += Lg
                tn = ep.tile([P, P], BF16, tag="tn_tile")
                nc.scalar.mul(tn[:], ps[:], disnd_sb[:, b:b + 1])
                if shard_next is not None:
                    ds = ep.tile([P, P], BF16, tag="ds_tile")
                    nc.scalar.mul(ds[:], tn[:], disnd_sb[:, b:b + 1])
                    nn = nbn(b)
                    nc.sync.dma_start(out=shard_next[b * P:b * P + nn, :],
                                      in_=ds[:nn, :])
                pt = tp.tile([P, P], BF16, tag="trb_ps")
                nc.tensor.transpose(pt[:], tn[:], idbf_sb[:])
                nc.scalar.copy(t_slab[:, b * P:(b + 1) * P], pt[:])

        def tag_out(w4, bias, slabs_in, slab_out):
            """slab_out <- gelu(sum_k W_k^T t_k + b)."""
            for b in range(NB):
                po = dp.tile([P, 512], F32, tag="dense_ps")
                for k in range(4):
                    nc.tensor.matmul(po[:, :P], lhsT=w4[k][:],
                                     rhs=slabs_in[k][:, b * P:(b + 1) * P],
                                     start=(k == 0), stop=(k == 3))
                nc.scalar.activation(slab_out[:, b * P:(b + 1) * P],
                                     po[:, :P],
                                     mybir.ActivationFunctionType.Gelu,
                                     bias=bias[:])

        def dense_inplace(w, bias, slab):
            for (o, cw) in chunks:
                ps = dp.tile([P, 512], F32, tag="dense_ps")
                nc.tensor.matmul(ps[:, :cw], lhsT=w[:], rhs=slab[:, o:o + cw],
                                 start=True, stop=True)
                nc.scalar.activation(slab[:, o:o + cw], ps[:, :cw],
                                     mybir.ActivationFunctionType.Gelu,
                                     bias=bias[:])

        # ================= TAG conv 1 =================
        write_ds0(slabA, shards[0])
        allgather(shards[0], tables[0])
        hop(tables[0], slabB, shards[1])
        allgather(shards[1], tables[1])
        hop(tables[1], slabC, shards[2])
        allgather(shards[2], tables[2])
        hop(tables[2], slabD, None)
        t1w = [wsb["tag1_w"][k] for k in range(4)]
        tag_out(t1w, wsb["tag1_b"], [slabA, slabB, slabC, slabD], slabB)

        # ================= middle linear =================
        dense_inplace(wsb["w2"], wsb["b2"], slabB)

        # ================= TAG conv 2 =================
        write_ds0(slabB, shards[3])
        allgather(shards[3], tables[3])
        hop(tables[3], slabA, shards[4])
        allgather(shards[4], tables[4])
        hop(tables[4], slabC, shards[5])
        allgather(shards[5], tables[5])
        hop(tables[5], slabD, None)
        t2w = [wsb["tag2_w"][k] for k in range(4)]
        tag_out(t2w, wsb["tag2_b"], [slabB, slabA, slabC, slabD], slabA)

        # ================= heads =================
        for b in range(NB):
            nn = nbn(b)
            xn = lp.tile([P, P], BF16, tag="xn_head")
            nc.sync.dma_start(out=xn[:], in_=xn_dram[:, b * P:(b + 1) * P])
            p5 = dp.tile([P, 512], F32, tag="dense_ps")
            nc.tensor.matmul(p5[:, :P], lhsT=wsb["w5a"][:], rhs=xn[:],
                             start=True, stop=False)
            nc.tensor.matmul(p5[:, :P], lhsT=wsb["w5b"][:],
                             rhs=slabA[:, b * P:(b + 1) * P],
                             start=False, stop=True)
            a1 = ep.tile([P, P], BF16, tag="a1")
            nc.scalar.activation(a1[:], p5[:, :P],
                                 mybir.ActivationFunctionType.Gelu,
                                 bias=wsb["b5"][:])
            p6 = tp.tile([P, P], F32, tag="tr_ps")
            nc.tensor.matmul(p6[:64, :P], lhsT=wsb["w6"][:], rhs=a1[:],
                             start=True, stop=True)
            a6 = ep.tile([64, P], F32, tag="a6")
            nc.scalar.activation(a6[:], p6[:64, :P],
                                 mybir.ActivationFunctionType.Identity,
                                 bias=wsb["b6"][:])
            g6 = ep.tile([64, P], BF16, tag="g6")
            nc.scalar.activation(g6[:], a6[:],
                                 mybir.ActivationFunctionType.Gelu)
            pc = tp.tile([P, P], F32, tag="tr_ps")
            nc.tensor.matmul(pc[:8, :P], lhsT=wsb["wc"][:], rhs=g6[:],
                             start=True, stop=True)
            cl_f = ep.tile([8, P], F32, tag="cl_f")
            nc.scalar.activation(cl_f[:], pc[:8, :P],
                                 mybir.ActivationFunctionType.Identity,
                                 bias=wsb["bc"][:])
            # softmax over the 64 features: transpose to node-major
            pa = tp.tile([P, P], F32, tag="tr_ps")
            nc.tensor.transpose(pa[:, :64], a6[:], idf32_sb[:64, :64])
            mx = ep.tile([P, 1], F32, tag="mx")
            nc.vector.tensor_reduce(out=mx[:], in_=pa[:, :64],
                                    axis=mybir.AxisListType.X,
                                    op=mybir.AluOpType.max)
            nmx = ep.tile([P, 1], F32, tag="nmx")
            nc.vector.tensor_scalar_mul(nmx[:], mx[:], -1.0)
            ex = ep.tile([P, 64], F32, tag="ex")
            nc.scalar.activation(ex[:], pa[:, :64],
                                 mybir.ActivationFunctionType.Exp,
                                 bias=nmx[:])
            sm = ep.tile([P, 1], F32, tag="sm")
            nc.vector.tensor_reduce(out=sm[:], in_=ex[:],
                                    axis=mybir.AxisListType.X,
                                    op=mybir.AluOpType.add)
            rs = ep.tile([P, 1], F32, tag="rs")
            nc.vector.reciprocal(rs[:], sm[:])
            ao = ep.tile([P, 64], F32, tag="ao")
            nc.vector.tensor_scalar_mul(ao[:], ex[:], rs[:])
            nc.sync.dma_start(out=a_out[b * P:b * P + nn, :], in_=ao[:nn, :])
            # cl to node-major
            pcl = tp.tile([P, P], F32, tag="tr_ps")
            nc.tensor.transpose(pcl[:, :8], cl_f[:], idf32_sb[:8, :8])
            co = ep.tile([P, 8], F32, tag="co")
            nc.vector.tensor_copy(out=co[:], in_=pcl[:, :8])
            nc.sync.dma_start(out=cl_out[b * P:b * P + nn, :], in_=co[:nn, :])

    nc.compile()
    return nc


# ---------------------------------------------------------------------------
# Host driver
# ---------------------------------------------------------------------------

def prep_inputs(inputs):
    x = np.asarray(inputs["x"], np.float32)
    ei = np.asarray(inputs["edge_index"])
    N = x.shape[0]
    cfg, gidx, selarr, dis_nd = prep_graph(ei, N)
    NCN = cfg["NCN"]

    idbf = np.eye(P, dtype=np.float32).astype(bf)
    idf32 = np.eye(P, dtype=np.float32)

    w5 = np.asarray(inputs["w5"], np.float32)
    com = {
        "idbf": idbf, "idf32": idf32,
        "w1": np.asarray(inputs["w1"]).astype(bf),
        "b1": np.asarray(inputs["b1"], np.float32).reshape(-1, 1),
        "tag1_w": np.asarray(inputs["tag1_w"]).astype(bf),
        "tag1_b": np.asarray(inputs["tag1_b"], np.float32).reshape(-1, 1),
        "w2": np.asarray(inputs["w2"]).astype(bf),
        "b2": np.asarray(inputs["b2"], np.float32).reshape(-1, 1),
        "tag2_w": np.asarray(inputs["tag2_w"]).astype(bf),
        "tag2_b": np.asarray(inputs["tag2_b"], np.float32).reshape(-1, 1),
        "w5a": w5[:P].astype(bf), "w5b": w5[P:].astype(bf),
        "b5": np.asarray(inputs["b5"], np.float32).reshape(-1, 1),
        "w6": np.asarray(inputs["w6"]).astype(bf),
        "b6": np.asarray(inputs["b6"], np.float32).reshape(-1, 1),
        "wc": np.asarray(inputs["wc"]).astype(bf),
        "bc": np.asarray(inputs["bc"], np.float32).reshape(-1, 1),
        "bn_gamma": np.asarray(inputs["bn_gamma"], np.float32).reshape(-1, 1),
        "bn_beta": np.asarray(inputs["bn_beta"], np.float32).reshape(-1, 1),
    }

    in_maps = []
    for c in range(CORES):
        m = dict(com)
        m["xT"] = np.ascontiguousarray(x[c * NCN:(c + 1) * NCN].T)
        m["gidx"] = gidx[c]
        m["sel"] = selarr[c]
        m["dis_nd"] = dis_nd[c]
        in_maps.append(m)
    return cfg, in_maps


def run(inputs, trace=False):
    cfg, in_maps = prep_inputs(inputs)
    nc = build_bass(cfg)
    res = run_bass_kernel_spmd(nc, in_maps, core_ids=list(range(CORES)),
                               trace=trace)
    a = np.concatenate([r["a_out"] for r in res.results], axis=0)
    cl = np.concatenate([r["cl_out"] for r in res.results], axis=0)
    return (a, cl), res


def kernel(**inputs):
    out, _ = run(inputs, trace=False)
    return out


# revision 24
# speedup vs baseline: 2.5547x; 1.0034x over previous
"""NodeGLAM (BN -> Linear -> TAGConv(K=3) -> Linear -> TAGConv(K=3) -> concat
-> MLP heads) on 8 Trainium2 NeuronCores.

Sharding: nodes (dim 0) split 8 ways; each core owns the edges whose
destination falls in its node range.  Per message-passing hop the cores
AllGather the (bf16) scaled activations, then use indirect DMA gathers +
one-hot selection matmuls to compute the segment sums for their own nodes.
Dense layers run feature-major (features on partitions, nodes on the free
axis) so weight matrices are used directly as the stationary matmul operand.
"""

import numpy as np
import ml_dtypes

import concourse.bass as bass
import concourse.bacc as bacc
import concourse.mybir as mybir
import concourse.tile as tile
from concourse.bass_utils import run_bass_kernel_spmd
from concourse.library_config import mlp as mlp_library

F32 = mybir.dt.float32
BF16 = mybir.dt.bfloat16
I16 = mybir.dt.int16

CORES = 8
P = 128
EPS = 1e-5

bf = ml_dtypes.bfloat16


def cdiv(a, b):
    return (a + b - 1) // b


def roundup(a, b):
    return cdiv(a, b) * b


# ---------------------------------------------------------------------------
# Host-side preprocessing: edge grouping / padding / index layout
# ---------------------------------------------------------------------------

def prep_graph(edge_index, N):
    """Group edges by (dest core, dest block of 128, source quarter), pad each
    group to a multiple of 128 slots (uniform across cores), and build the
    per-core gather-index / local-column arrays in the layouts the device
    kernel consumes."""
    NCN = N // CORES            # nodes per core
    NB = cdiv(NCN, P)           # dest blocks per core
    QR = N // 4                 # rows per source quarter (int16 index range)
    assert QR <= 32767

    row = edge_index[0].astype(np.int64)
    col = edge_index[1].astype(np.int64)
    E = row.shape[0]

    deg = np.bincount(col, minlength=N)
    dis = np.where(deg > 0, 1.0 / np.sqrt(np.maximum(deg, 1.0)), 0.0).astype(
        np.float32
    )

    core = col // NCN
    lcol = col % NCN
    blk = lcol // P
    lc = lcol % P
    q = row // QR
    lr = row % QR

    G = NB * 4
    gkey = (core * G) + blk * 4 + q
    order = np.argsort(gkey, kind="stable")
    gkey_s = gkey[order]
    counts = np.bincount(gkey, minlength=CORES * G).reshape(CORES, NB, 4)

    Lmax = counts.max(axis=0)                       # [NB, 4]
    Lbq = (np.ceil(Lmax / P).astype(np.int64) * P)  # 128-padded (sel/lc/tiles)
    # Gather counts must cover the full 128-padded tile: slots never written
    # by any gather would otherwise hold stale SBUF garbage, and 0 * NaN
    # poisons the selection matmul.
    L16 = Lbq.copy()
    assert (counts <= L16[None]).all() and (L16 <= Lbq).all()

    offs = np.zeros(G + 1, np.int64)
    offs[1:] = np.cumsum(Lbq.reshape(-1))
    S = int(offs[-1])
    offs16 = np.zeros(G + 1, np.int64)
    offs16[1:] = np.cumsum(L16.reshape(-1))
    S16 = int(offs16[-1])
    assert S % P == 0 and S > 0 and S16 % 16 == 0

    # slot position of every edge
    starts = np.zeros(CORES * G + 1, np.int64)
    starts[1:] = np.cumsum(counts.reshape(-1))
    rank = np.arange(E) - np.repeat(starts[:-1], counts.reshape(-1))
    core_s = core[order]
    g_within = gkey_s % G

    # gather indices: packed at 16-padded offsets
    lr_slots = np.zeros((CORES, S16), np.int16)
    lr_slots[core_s, offs16[g_within] + rank] = lr[order].astype(np.int16)
    # local dest columns: packed at 128-padded offsets (200 => one-hot row 0)
    lc_slots = np.full((CORES, S), 200.0, np.float32)
    lc_slots[core_s, offs[g_within] + rank] = lc[order].astype(np.float32)

    # gather-index layout: idx j -> partition j%16, column j//16, replicated 8x
    gidx = lr_slots.reshape(CORES, S16 // 16, 16).transpose(0, 2, 1)
    gidx = np.ascontiguousarray(np.tile(gidx, (1, 8, 1)))   # [CORES,128,S16/16]
    # selection one-hots in gather-output layout: slot j -> partition j%128,
    # tile j//128; sel[core, p, c*128:(c+1)*128] = onehot(lc[slot c*128+p])
    lcarr = lc_slots.reshape(CORES, S // P, P).transpose(0, 2, 1)
    sel = (lcarr[:, :, :, None] == np.arange(P, dtype=np.float32)).astype(bf)
    sel = np.ascontiguousarray(sel.reshape(CORES, P, S))    # [CORES,128,S]

    # dis node-major per block: [CORES, 128, NB]
    dn = np.zeros((CORES, NB * P), np.float32)
    dn[:, :NCN] = dis.reshape(CORES, NCN)
    dis_nd = np.ascontiguousarray(dn.reshape(CORES, NB, P).transpose(0, 2, 1))

    cfg = dict(
        N=N, E=E, NCN=NCN, NB=NB, QR=QR, S=S, S16=S16,
        L=Lbq.astype(int).tolist(), L16=L16.astype(int).tolist(),
    )
    return cfg, gidx, sel, dis_nd


# ---------------------------------------------------------------------------
# Bass program
# ---------------------------------------------------------------------------

def build_bass(cfg):
    N = cfg["N"]
    NCN = cfg["NCN"]
    NB = cfg["NB"]
    QR = cfg["QR"]
    S = cfg["S"]
    S16 = cfg["S16"]
    L = cfg["L"]          # [NB][4] padded group sizes (multiples of 128, or 0)
    L16 = cfg["L16"]      # [NB][4] 16-padded gather counts
    NBP = NB * P
    NQUEUE = 4

    nc = bacc.Bacc("TRN2", target_bir_lowering=False, num_devices=CORES,
                   num_swdge_queues=NQUEUE)
    rg = [list(range(CORES))]

    # ---------------- kernel I/O ----------------
    xT = nc.dram_tensor("xT", [P, NCN], F32, kind="ExternalInput")
    gidx_d = nc.dram_tensor("gidx", [P, S16 // 16], I16, kind="ExternalInput")
    sel_d = nc.dram_tensor("sel", [P, S], BF16, kind="ExternalInput")
    idbf_d = nc.dram_tensor("idbf", [P, P], BF16, kind="ExternalInput")
    idf32_d = nc.dram_tensor("idf32", [P, P], F32, kind="ExternalInput")
    disnd_d = nc.dram_tensor("dis_nd", [P, NB], F32, kind="ExternalInput")

    wname = [
        ("w1", [P, P], BF16), ("b1", [P, 1], F32),
        ("tag1_w", [4, P, P], BF16), ("tag1_b", [P, 1], F32),
        ("w2", [P, P], BF16), ("b2", [P, 1], F32),
        ("tag2_w", [4, P, P], BF16), ("tag2_b", [P, 1], F32),
        ("w5a", [P, P], BF16), ("w5b", [P, P], BF16), ("b5", [P, 1], F32),
        ("w6", [P, 64], BF16), ("b6", [64, 1], F32),
        ("wc", [64, 8], BF16), ("bc", [8, 1], F32),
        ("bn_gamma", [P, 1], F32), ("bn_beta", [P, 1], F32),
    ]
    wd = {nm: nc.dram_tensor(nm, sh, dt, kind="ExternalInput")
          for nm, sh, dt in wname}

    a_out = nc.dram_tensor("a_out", [NCN, 64], F32, kind="ExternalOutput")
    cl_out = nc.dram_tensor("cl_out", [NCN, 8], F32, kind="ExternalOutput")

    # ---------------- internal DRAM ----------------
    xn_dram = nc.dram_tensor("xn_dram", [P, NBP], BF16, kind="Internal")
    shards = [nc.dram_tensor(f"shard{h}", [NCN, P], BF16, kind="Internal")
              for h in range(6)]
    tables = [nc.dram_tensor(f"table{h}", [N, P], BF16, kind="Internal",
                             addr_space="Shared")
              for h in range(6)]
    bn_in = nc.dram_tensor("bn_in", [P, 2], F32, kind="Internal")
    bn_out = nc.dram_tensor("bn_out", [P, 2], F32, kind="Internal",
                            addr_space="Shared")

    def nbn(b):                      # valid node count of block b
        return min(P, NCN - b * P)

    from contextlib import ExitStack
    with tile.TileContext(nc) as tc, ExitStack() as ctx:
        nc.gpsimd.load_library(mlp_library)

        cpool = ctx.enter_context(tc.tile_pool(name="consts", bufs=1))
        slabp = ctx.enter_context(tc.tile_pool(name="slabs", bufs=1))
        wpool = ctx.enter_context(tc.tile_pool(name="weights", bufs=1))
        lp = ctx.enter_context(tc.tile_pool(name="loads", bufs=2))
        gp = ctx.enter_context(tc.tile_pool(name="gather", bufs=12))
        sp = ctx.enter_context(tc.tile_pool(name="sel", bufs=12))
        ep = ctx.enter_context(tc.tile_pool(name="evac", bufs=4))
        pp = ctx.enter_context(tc.tile_pool(name="psum", bufs=3, space="PSUM"))
        tp = ctx.enter_context(tc.tile_pool(name="tpsum", bufs=2, space="PSUM"))
        dp = ctx.enter_context(tc.tile_pool(name="dpsum", bufs=2, space="PSUM"))
        stat = ctx.enter_context(tc.tile_pool(name="stats", bufs=1))

        # ---- constants into SBUF ----
        zero_c = cpool.tile([P, 1], F32, tag="zero_c")
        nc.vector.memset(zero_c[:], 0.0)
        nc.const_aps.aps[(F32, 0.0)] = zero_c[:]
        eps_c = cpool.tile([P, 1], F32, tag="eps_c")
        nc.vector.memset(eps_c[:], EPS)

        gidx_sb = cpool.tile([P, S16 // 16], I16, tag="gidx_sb")
        nc.sync.dma_start(out=gidx_sb[:], in_=gidx_d[:])
        idbf_sb = cpool.tile([P, P], BF16, tag="idbf_sb")
        nc.sync.dma_start(out=idbf_sb[:], in_=idbf_d[:])
        idf32_sb = cpool.tile([P, P], F32, tag="idf32_sb")
        nc.sync.dma_start(out=idf32_sb[:], in_=idf32_d[:])
        disnd_sb = cpool.tile([P, NB], F32, tag="disnd_sb")
        nc.sync.dma_start(out=disnd_sb[:], in_=disnd_d[:])

        wsb = {}
        for nm, sh, dt in wname:
            if nm in ("tag1_w", "tag2_w"):
                tl = []
                for k in range(4):
                    t = wpool.tile([P, P], dt, tag=f"w_{nm}{k}")
                    nc.sync.dma_start(out=t[:], in_=wd[nm][k])
                    tl.append(t)
                wsb[nm] = tl
            else:
                t = wpool.tile(sh, dt, tag=f"w_{nm}")
                nc.sync.dma_start(out=t[:], in_=wd[nm][:])
                wsb[nm] = t

        # activation slabs, feature-major [128 feats, NBP nodes] bf16
        slabA = slabp.tile([P, NBP], BF16, tag="slabA")
        slabB = slabp.tile([P, NBP], BF16, tag="slabB")
        slabC = slabp.tile([P, NBP], BF16, tag="slabC")
        slabD = slabp.tile([P, NBP], BF16, tag="slabD")

        # ---- node chunks for the dense stages ----
        chunks = []
        c0 = 0
        while c0 < NCN:
            cw = min(512, NCN - c0)
            chunks.append((c0, cw))
            c0 += cw

        # ================= BatchNorm statistics =================
        nch = len(chunks)
        sums = stat.tile([P, nch], F32, tag="sums")
        sqs = stat.tile([P, nch], F32, tag="sqs")
        for ci, (o, cw) in enumerate(chunks):
            xs = lp.tile([P, 512], F32, tag="xchunk")
            nc.sync.dma_start(out=xs[:, :cw], in_=xT[:, o:o + cw])
            nc.vector.tensor_reduce(
                out=sums[:, ci:ci + 1], in_=xs[:, :cw],
                axis=mybir.AxisListType.X, op=mybir.AluOpType.add)
            x2 = lp.tile([P, 512], F32, tag="x2chunk")
            nc.scalar.square(x2[:, :cw], xs[:, :cw])
            nc.vector.tensor_reduce(
                out=sqs[:, ci:ci + 1], in_=x2[:, :cw],
                axis=mybir.AxisListType.X, op=mybir.AluOpType.add)
        tot = stat.tile([P, 2], F32, tag="tot")
        nc.vector.tensor_reduce(out=tot[:, 0:1], in_=sums[:],
                                axis=mybir.AxisListType.X,
                                op=mybir.AluOpType.add)
        nc.vector.tensor_reduce(out=tot[:, 1:2], in_=sqs[:],
                                axis=mybir.AxisListType.X,
                                op=mybir.AluOpType.add)
        nc.sync.dma_start(out=bn_in[:], in_=tot[:])
        nc.gpsimd.collective_compute(
            "AllReduce", mybir.AluOpType.add, replica_groups=rg,
            ins=[bn_in[:]], outs=[bn_out[:]])
        tot2 = stat.tile([P, 2], F32, tag="tot2")
        nc.sync.dma_start(out=tot2[:], in_=bn_out[:])

        mu = stat.tile([P, 1], F32, tag="mu")
        nc.scalar.mul(mu[:], tot2[:, 0:1], 1.0 / N)
        msq = stat.tile([P, 1], F32, tag="msq")
        nc.scalar.mul(msq[:], tot2[:, 1:2], 1.0 / N)
        mu2 = stat.tile([P, 1], F32, tag="mu2")
        nc.vector.tensor_tensor(out=mu2[:], in0=mu[:], in1=mu[:],
                                op=mybir.AluOpType.mult)
        var = stat.tile([P, 1], F32, tag="var")
        nc.vector.tensor_tensor(out=var[:], in0=msq[:], in1=mu2[:],
                                op=mybir.AluOpType.subtract)
        sd = stat.tile([P, 1], F32, tag="sd")
        nc.scalar.activation(sd[:], var[:], mybir.ActivationFunctionType.Sqrt,
                             bias=eps_c[:])
        inv = stat.tile([P, 1], F32, tag="inv")
        nc.vector.reciprocal(inv[:], sd[:])
        scale = stat.tile([P, 1], F32, tag="scale")
        nc.vector.tensor_tensor(out=scale[:], in0=inv[:], in1=wsb["bn_gamma"][:],
                                op=mybir.AluOpType.mult)
        mus = stat.tile([P, 1], F32, tag="mus")
        nc.vector.tensor_tensor(out=mus[:], in0=mu[:], in1=scale[:],
                                op=mybir.AluOpType.mult)
        shift = stat.tile([P, 1], F32, tag="shift")
        nc.vector.tensor_tensor(out=shift[:], in0=wsb["bn_beta"][:], in1=mus[:],
                                op=mybir.AluOpType.subtract)

        # ============ normalize + first linear (h1 = gelu(xn@w1+b1)) ============
        for (o, cw) in chunks:
            xs = lp.tile([P, 512], F32, tag="xchunk")
            nc.sync.dma_start(out=xs[:, :cw], in_=xT[:, o:o + cw])
            xn = lp.tile([P, 512], BF16, tag="xnchunk")
            nc.scalar.activation(xn[:, :cw], xs[:, :cw],
                                 mybir.ActivationFunctionType.Identity,
                                 bias=shift[:], scale=scale[:])
            nc.sync.dma_start(out=xn_dram[:, o:o + cw], in_=xn[:, :cw])
            ps = dp.tile([P, 512], F32, tag="dense_ps")
            nc.tensor.matmul(ps[:, :cw], lhsT=wsb["w1"][:], rhs=xn[:, :cw],
                             start=True, stop=True)
            nc.scalar.activation(slabA[:, o:o + cw], ps[:, :cw],
                                 mybir.ActivationFunctionType.Gelu,
                                 bias=wsb["b1"][:])

        # ================= helpers =================
        def write_ds0(h_slab, shard):
            """shard <- dis * h  (node-major), from feature-major slab."""
            for b in range(NB):
                pt = tp.tile([P, P], BF16, tag="trb_ps")
                nc.tensor.transpose(pt[:], h_slab[:, b * P:(b + 1) * P],
                                    idbf_sb[:])
                ds = ep.tile([P, P], BF16, tag="ds_tile")
                nc.scalar.mul(ds[:], pt[:], disnd_sb[:, b:b + 1])
                nn = nbn(b)
                nc.sync.dma_start(out=shard[b * P:b * P + nn, :],
                                  in_=ds[:nn, :])

        def allgather(shard, table):
            nc.gpsimd.collective_compute(
                "AllGather", mybir.AluOpType.bypass, replica_groups=rg,
                ins=[shard[:]], outs=[table[:]])

        qctr = [0]

        def hop(table, t_slab, shard_next):
            """t_slab <- dis * (segment-sum over edges of table[row]);
            shard_next (if not None) <- dis * t_slab  (node-major)."""
            for b in range(NB):
                ntot = sum(L[b][q] for q in range(4)) // P
                assert ntot > 0
                ps = pp.tile([P, P], F32, tag="scat_ps")
                mm = 0
                off = sum(sum(L[bb]) for bb in range(b))
                off16 = sum(sum(L16[bb]) for bb in range(b))
                for q in range(4):
                    Lq = L[b][q]
                    Lg = L16[b][q]
                    if Lq == 0:
                        continue
                    nt = Lq // P
                    g = gp.tile([P, 8, P], BF16, tag="gath")
                    nc.gpsimd.dma_gather(
                        out_ap=g[:, :nt, :],
                        in_ap=table[q * QR:(q + 1) * QR, :],
                        idxs_ap=gidx_sb[:, off16 // 16:(off16 + Lg) // 16],
                        num_idxs=Lg, num_idxs_reg=Lg, elem_size=P,
                        queue_num=qctr[0] % NQUEUE)
                    qctr[0] += 1
                    sel = sp.tile([P, 8 * P], BF16, tag="sel")
                    nc.sync.dma_start(out=sel[:, :Lq],
                                      in_=sel_d[:, off:off + Lq])
                    for t in range(nt):
                        nc.tensor.matmul(ps[:], lhsT=sel[:, t * P:(t + 1) * P],
                                         rhs=g[:, t, :],
                                         start=(mm == 0), stop=(mm == ntot - 1))
                        mm += 1
                    off += Lq
                    off16 += Lg
                tn = ep.tile([P, P], BF16, tag="tn_tile")
                nc.scalar.mul(tn[:], ps[:], disnd_sb[:, b:b + 1])
                if shard_next is not None:
                    ds = ep.tile([P, P], BF16, tag="ds_tile")
                    nc.scalar.mul(ds[:], tn[:], disnd_sb[:, b:b + 1])
                    nn = nbn(b)
                    nc.sync.dma_start(out=shard_next[b * P:b * P + nn, :],
                                      in_=ds[:nn, :])
                pt = tp.tile([P, P], BF16, tag="trb_ps")
                nc.tensor.transpose(pt[:], tn[:], idbf_sb[:])
                nc.scalar.copy(t_slab[:, b * P:(b + 1) * P], pt[:])

        def tag_out(w4, bias, slabs_in, slab_out):
            """slab_out <- gelu(sum_k W_k^T t_k + b)."""
            for b in range(NB):
                po = dp.tile([P, 512], F32, tag="dense_ps")
                for k in range(4):
                    nc.tensor.matmul(po[:, :P], lhsT=w4[k][:],
                                     rhs=slabs_in[k][:, b * P:(b + 1) * P],
                                     start=(k == 0), stop=(k == 3))
                nc.scalar.activation(slab_out[:, b * P:(b + 1) * P],
                                     po[:, :P],
                                     mybir.ActivationFunctionType.Gelu,
                                     bias=bias[:])

        def dense_inplace(w, bias, slab):
            for (o, cw) in chunks:
                ps = dp.tile([P, 512], F32, tag="dense_ps")
                nc.tensor.matmul(ps[:, :cw], lhsT=w[:], rhs=slab[:, o:o + cw],
                                 start=True, stop=True)
                nc.scalar.activation(slab[:, o:o + cw], ps[:, :cw],
                                     mybir.ActivationFunctionType.Gelu,
                                     bias=bias[:])

        # ================= TAG conv 1 =================
        write_ds0(slabA, shards[0])
        allgather(shards[0], tables[0])
        hop(tables[0], slabB, shards[1])
        allgather(shards[1], tables[1])
        hop(tables[1], slabC, shards[2])
        allgather(shards[2], tables[2])
        hop(tables[2], slabD, None)
        t1w = [wsb["tag1_w"][k] for k in range(4)]
        tag_out(t1w, wsb["tag1_b"], [slabA, slabB, slabC, slabD], slabB)

        # ================= middle linear =================
        dense_inplace(wsb["w2"], wsb["b2"], slabB)

        # ================= TAG conv 2 =================
        write_ds0(slabB, shards[3])
        allgather(shards[3], tables[3])
        hop(tables[3], slabA, shards[4])
        allgather(shards[4], tables[4])
        hop(tables[4], slabC, shards[5])
        allgather(shards[5], tables[5])
        hop(tables[5], slabD, None)
        t2w = [wsb["tag2_w"][k] for k in range(4)]
        tag_out(t2w, wsb["tag2_b"], [slabB, slabA, slabC, slabD], slabA)

        # ================= heads =================
        for b in range(NB):
            nn = nbn(b)
            xn = lp.tile([P, P], BF16, tag="xn_head")
            nc.sync.dma_start(out=xn[:], in_=xn_dram[:, b * P:(b + 1) * P])
            p5 = dp.tile([P, 512], F32, tag="dense_ps")
            nc.tensor.matmul(p5[:, :P], lhsT=wsb["w5a"][:], rhs=xn[:],
                             start=True, stop=False)
            nc.tensor.matmul(p5[:, :P], lhsT=wsb["w5b"][:],
                             rhs=slabA[:, b * P:(b + 1) * P],
                             start=False, stop=True)
            a1 = ep.tile([P, P], BF16, tag="a1")
            nc.scalar.activation(a1[:], p5[:, :P],
                                 mybir.ActivationFunctionType.Gelu,
                                 bias=wsb["b5"][:])
            p6 = tp.tile([P, P], F32, tag="trb_ps")
            nc.tensor.matmul(p6[:64, :P], lhsT=wsb["w6"][:], rhs=a1[:],
                             start=True, stop=True)
            a6 = ep.tile([64, P], F32, tag="a6")
            nc.scalar.activation(a6[:], p6[:64, :P],
                                 mybir.ActivationFunctionType.Identity,
                                 bias=wsb["b6"][:])
            g6 = ep.tile([64, P], BF16, tag="g6")
            nc.scalar.activation(g6[:], a6[:],
                                 mybir.ActivationFunctionType.Gelu)
            pc = tp.tile([P, P], F32, tag="trb_ps")
            nc.tensor.matmul(pc[:8, :P], lhsT=wsb["wc"][:], rhs=g6[:],
                             start=True, stop=True)
            cl_f = ep.tile([8, P], F32, tag="cl_f")
            nc.scalar.activation(cl_f[:], pc[:8, :P],
                                 mybir.ActivationFunctionType.Identity,
                                 bias=wsb["bc"][:])
            # softmax over the 64 features: transpose to node-major
            pa = tp.tile([P, P], F32, tag="trb_ps")
            nc.tensor.transpose(pa[:, :64], a6[:], idf32_sb[:64, :64])
            mx = ep.tile([P, 1], F32, tag="mx")
            nc.vector.tensor_reduce(out=mx[:], in_=pa[:, :64],
                                    axis=mybir.AxisListType.X,
                                    op=mybir.AluOpType.max)
            nmx = ep.tile([P, 1], F32, tag="nmx")
            nc.vector.tensor_scalar_mul(nmx[:], mx[:], -1.0)
            ex = ep.tile([P, 64], F32, tag="ex")
            nc.scalar.activation(ex[:], pa[:, :64],
                                 mybir.ActivationFunctionType.Exp,
                                 bias=nmx[:])
            sm = ep.tile([P, 1], F32, tag="sm")
            nc.vector.tensor_reduce(out=sm[:], in_=ex[:],
                                    axis=mybir.AxisListType.X,
                                    op=mybir.AluOpType.add)
            rs = ep.tile([P, 1], F32, tag="rs")
            nc.vector.reciprocal(rs[:], sm[:])
            ao = ep.tile([P, 64], F32, tag="ao")
            nc.vector.tensor_scalar_mul(ao[:], ex[:], rs[:])
            nc.sync.dma_start(out=a_out[b * P:b * P + nn, :], in_=ao[:nn, :])
            # cl to node-major
            pcl = tp.tile([P, P], F32, tag="trb_ps")
            nc.tensor.transpose(pcl[:, :8], cl_f[:], idf32_sb[:8, :8])
            co = ep.tile([P, 8], F32, tag="co")
            nc.vector.tensor_copy(out=co[:], in_=pcl[:, :8])
            nc.sync.dma_start(out=cl_out[b * P:b * P + nn, :], in_=co[:nn, :])

    nc.compile()
    return nc


# ---------------------------------------------------------------------------
# Host driver
# ---------------------------------------------------------------------------

def prep_inputs(inputs):
    x = np.asarray(inputs["x"], np.float32)
    ei = np.asarray(inputs["edge_index"])
    N = x.shape[0]
    cfg, gidx, selarr, dis_nd = prep_graph(ei, N)
    NCN = cfg["NCN"]

    idbf = np.eye(P, dtype=np.float32).astype(bf)
    idf32 = np.eye(P, dtype=np.float32)

    w5 = np.asarray(inputs["w5"], np.float32)
    com = {
        "idbf": idbf, "idf32": idf32,
        "w1": np.asarray(inputs["w1"]).astype(bf),
        "b1": np.asarray(inputs["b1"], np.float32).reshape(-1, 1),
        "tag1_w": np.asarray(inputs["tag1_w"]).astype(bf),
        "tag1_b": np.asarray(inputs["tag1_b"], np.float32).reshape(-1, 1),
        "w2": np.asarray(inputs["w2"]).astype(bf),
        "b2": np.asarray(inputs["b2"], np.float32).reshape(-1, 1),
        "tag2_w": np.asarray(inputs["tag2_w"]).astype(bf),
        "tag2_b": np.asarray(inputs["tag2_b"], np.float32).reshape(-1, 1),
        "w5a": w5[:P].astype(bf), "w5b": w5[P:].astype(bf),
        "b5": np.asarray(inputs["b5"], np.float32).reshape(-1, 1),
        "w6": np.asarray(inputs["w6"]).astype(bf),
        "b6": np.asarray(inputs["b6"], np.float32).reshape(-1, 1),
        "wc": np.asarray(inputs["wc"]).astype(bf),
        "bc": np.asarray(inputs["bc"], np.float32).reshape(-1, 1),
        "bn_gamma": np.asarray(inputs["bn_gamma"], np.float32).reshape(-1, 1),
        "bn_beta": np.asarray(inputs["bn_beta"], np.float32).reshape(-1, 1),
    }

    in_maps = []
    for c in range(CORES):
        m = dict(com)
        m["xT"] = np.ascontiguousarray(x[c * NCN:(c + 1) * NCN].T)
        m["gidx"] = gidx[c]
        m["sel"] = selarr[c]
        m["dis_nd"] = dis_nd[c]
        in_maps.append(m)
    return cfg, in_maps


def run(inputs, trace=False):
    cfg, in_maps = prep_inputs(inputs)
    nc = build_bass(cfg)
    res = run_bass_kernel_spmd(nc, in_maps, core_ids=list(range(CORES)),
                               trace=trace)
    a = np.concatenate([r["a_out"] for r in res.results], axis=0)
    cl = np.concatenate([r["cl_out"] for r in res.results], axis=0)
    return (a, cl), res


def kernel(**inputs):
    out, _ = run(inputs, trace=False)
    return out
